# revision 1
# baseline (speedup 1.0000x reference)
"""Trainium2 Bass kernel for nn_Counting: per-batch l2-normalize ->
self-similarity gram -> relu row-sum counter -> softplus expander ->
concat-merger dense.

Sharding: data-parallel over batch. B=8 batch elements across 8 cores,
weights replicated. Each core runs the identical single-core program on
its [2048, 1024] slice.

Per-core math (N=2048, D=1024):
  sq_n   = sum_d x[n,d]^2 ;  r_n = rsqrt(sq_n) = exp(-0.5*ln(sq_n)) ; s_n = 1/r_n
  normed = x * r            (tensor_scalar, natural layout)
  normedT[d, n]             (PE transposes, f32r-rounded on the PSUM->SBUF copy)
  G[n, m] = normed_n . normed_m    (fp32r matmuls, K=D on partitions)
  counter_n = sum_m relu(G[n, m])  (ACT relu + accum_out row sums)
  cspT[dd, n] = softplus(W1[dd]*counter_n + b1[dd]) = ln(1 + exp(.))
                built in two n-halves overlapped with the sim matmuls
                (counter -> PE transpose -> row -> DRAM bounce -> bcast ->
                 ACT exp with per-partition scale/bias -> ACT ln bias=1)
  out = s .* (normed @ W2a) + csp @ W2b
        (two PSUM accumulations; A-term scaled back by s_n so the merger
         uses normedT as lhsT without materializing dataT)
"""

import numpy as np
import orjson

import concourse.bass as bass
import concourse.mybir as mybir
import concourse.tile as tile
from concourse.masks import make_identity
from concourse.bass_utils import run_bass_kernel_spmd

F32 = mybir.dt.float32
F32R = mybir.dt.float32r
BF16 = mybir.dt.bfloat16
AF = mybir.ActivationFunctionType
ALU = mybir.AluOpType

B, N, D = 8, 2048, 1024
NT = N // 128   # 16 n-tiles
KD = D // 128   # 8 d-chunks
MJ = N // 512   # 4 m-chunks of 512

_MAX_WAITS = 1


def _legalize_bir_waits(bir_bytes: bytes) -> bytes:
    """This walrus build accepts very few sync-wait commands per instruction
    (1 for S3_LW matmuls, <3 for Drain). Tile freely attaches several. Hoist
    extra waits onto standalone Drains inserted before the instruction on the
    same engine (engine program order keeps semantics identical)."""
    d = orjson.loads(bir_bytes)
    n_new = 0
    for fn in d.get("functions", []):
        for blk in fn.get("blocks", []):
            out = []
            changed = False
            for inst in blk.get("instructions", []):
                si = inst.get("sync_info")
                waits = (si or {}).get("on_wait") or []
                if len(waits) > _MAX_WAITS:
                    extra, keep = waits[:-_MAX_WAITS], waits[-_MAX_WAITS:]
                    for w in extra:
                        n_new += 1
                        out.append({
                            "debug": inst.get("debug"),
                            "engine": inst["engine"],
                            "ins": [], "outs": [],
                            "is_reset_sema": False,
                            "name": f"waitfix-{n_new}",
                            "opcode": "NoOp",
                            "sync_info": {"on_update": [], "on_wait": [w]},
                        })
                    si["on_wait"] = keep
                    changed = True
                out.append(inst)
            if changed:
                blk["instructions"] = out
    return orjson.dumps(d)


def _install_waitfix():
    import concourse.bass_utils as bu
    import concourse.bass2jax as b2j

    if getattr(bu.compile_bir_kernel, "_waitfix", False):
        return
    orig = bu.compile_bir_kernel

    def patched(bir_json, tmpdir, *args, **kwargs):
        if isinstance(bir_json, str):
            bir_json = bir_json.encode()
        return orig(_legalize_bir_waits(bir_json), tmpdir, *args, **kwargs)

    patched._waitfix = True
    bu.compile_bir_kernel = patched
    b2j.compile_bir_kernel = patched


def build_kernel(repeat: int = 1):
    nc = bass.Bass(trn_type="TRN2")
    data = nc.dram_tensor("data", [N, D], F32, kind="ExternalInput")
    W1 = nc.dram_tensor("W1", [1, D], F32, kind="ExternalInput")
    b1 = nc.dram_tensor("b1", [1, D], F32, kind="ExternalInput")
    W2 = nc.dram_tensor("W2", [2 * D, D], F32, kind="ExternalInput")
    out = nc.dram_tensor("out", [N, D], F32, kind="ExternalOutput")
    row_scratch = nc.dram_tensor("row_scratch", [1, N], F32)

    with tile.TileContext(nc) as tc:
        with (
            tc.tile_pool(name="big", bufs=1) as big,
            tc.tile_pool(name="xp", bufs=3) as xp,
            tc.tile_pool(name="w2tmp", bufs=2) as w2tmp,
            tc.tile_pool(name="small", bufs=1) as small,
            tc.tile_pool(name="outp", bufs=2) as outp,
            tc.tile_pool(name="t1p", bufs=2) as t1p,
            tc.tile_pool(name="ps_tp", bufs=2, space="PSUM") as ps_tp,
            tc.tile_pool(name="ps_g", bufs=2, space="PSUM") as ps_g,
            tc.tile_pool(name="ps_a", bufs=2, space="PSUM") as ps_a,
            tc.tile_pool(name="ps_b", bufs=2, space="PSUM") as ps_b,
        ):
            # ---- resident tensors
            normedT = big.tile([128, KD, N], F32R)     # 64KB/part
            w2a = big.tile([128, KD, D], F32R)         # 32KB/part
            w2b = big.tile([128, KD, D], BF16)         # 16KB/part
            cspT = big.tile([128, KD, N], BF16)        # 32KB/part
            bc = big.tile([128, N], F32)               # 8KB/part
            exp_scr = big.tile([128, N // 2], F32)     # 4KB/part
            relu_scr = big.tile([128, 512], F32)       # 2KB/part
            sq_scr = big.tile([128, D], F32)           # 4KB/part

            ident = small.tile([128, 128], F32)
            make_identity(nc, ident)
            W1T = small.tile([128, KD], F32)
            b1T = small.tile([128, KD], F32)
            sq_all = small.tile([128, NT], F32)
            lnsq = small.tile([128, NT], F32)
            r_all = small.tile([128, NT], F32)
            s_all = small.tile([128, NT], F32)
            counter_all = small.tile([128, NT], F32)
            cpart = small.tile([128, NT * MJ], F32)
            counterT = small.tile([8, 128], F32)
            counter_row = small.tile([1, N], F32)

            def body(it):
                nc.sync.dma_start(
                    out=W1T[:, :],
                    in_=bass.AP(tensor=W1, offset=0, ap=[[1, 128], [128, KD]]),
                )
                nc.sync.dma_start(
                    out=b1T[:, :],
                    in_=bass.AP(tensor=b1, offset=0, ap=[[1, 128], [128, KD]]),
                )

                # ---- stage A: load, norms, normed, transpose
                for i in range(NT):
                    X = xp.tile([128, D], F32, tag="X")
                    nc.sync.dma_start(out=X, in_=data[128 * i:128 * (i + 1), :])
                    nc.scalar.activation(out=sq_scr, in_=X, func=AF.Square,
                                         accum_out=sq_all[:, i:i + 1])
                    nc.scalar.activation(out=lnsq[:, i:i + 1],
                                         in_=sq_all[:, i:i + 1], func=AF.Ln)
                    nc.scalar.activation(out=r_all[:, i:i + 1],
                                         in_=lnsq[:, i:i + 1], func=AF.Exp,
                                         scale=-0.5)
                    nc.scalar.activation(out=s_all[:, i:i + 1],
                                         in_=lnsq[:, i:i + 1], func=AF.Exp,
                                         scale=0.5)
                    nc.vector.tensor_scalar_mul(out=X, in0=X,
                                                scalar1=r_all[:, i:i + 1])
                    for g in range(2):
                        tp = ps_tp.tile([128, 512], F32, tag="tp")
                        for k in range(4):
                            nc.tensor.transpose(
                                tp[:, 128 * k:128 * (k + 1)],
                                X[:, 512 * g + 128 * k: 512 * g + 128 * (k + 1)],
                                ident[:, :],
                            )
                        nc.vector.tensor_copy(
                            normedT[:, 4 * g:4 * (g + 1), 128 * i:128 * (i + 1)],
                            tp[:, :].rearrange("p (c n) -> p c n", c=4),
                        )

                # ---- W2 load + cast (after stage A so data DMAs go first)
                for c in range(KD):
                    t = w2tmp.tile([128, D], F32, tag="w2tmp")
                    nc.sync.dma_start(out=t, in_=W2[128 * c:128 * (c + 1), :])
                    nc.scalar.copy(out=w2a[:, c, :], in_=t)
                for c in range(KD):
                    t = w2tmp.tile([128, D], F32, tag="w2tmp")
                    nc.sync.dma_start(out=t,
                                      in_=W2[D + 128 * c:D + 128 * (c + 1), :])
                    nc.scalar.copy(out=w2b[:, c, :], in_=t)

                def csp_half(h):
                    # counter cols [8h, 8h+8) -> cspT[:, :, 1024h : 1024h+1024]
                    tpc = ps_tp.tile([8, 128], F32, tag="tp")
                    nc.tensor.transpose(tpc, counter_all[:, 8 * h:8 * (h + 1)],
                                        ident[:, :])
                    nc.vector.tensor_copy(counterT, tpc)
                    half = slice(1024 * h, 1024 * (h + 1))
                    nc.sync.dma_start(out=counter_row[:, half],
                                      in_=counterT[:, :])
                    nc.sync.dma_start(out=row_scratch[:, half],
                                      in_=counter_row[:, half])
                    nc.sync.dma_start(
                        out=bc[:, half],
                        in_=bass.AP(tensor=row_scratch, offset=1024 * h,
                                    ap=[[0, 128], [1, 1024]]),
                    )
                    for kd in range(KD):
                        nc.scalar.activation(out=exp_scr, in_=bc[:, half],
                                             func=AF.Exp,
                                             bias=b1T[:, kd:kd + 1],
                                             scale=W1T[:, kd:kd + 1])
                        nc.scalar.activation(out=cspT[:, kd, half],
                                             in_=exp_scr, func=AF.Ln, bias=1.0)

                # ---- stage B: gram + relu row-sums (+ csp halves interleaved)
                for i in range(NT):
                    for j in range(MJ):
                        G = ps_g.tile([128, 512], F32, tag="G")
                        for kd in range(KD):
                            nc.tensor.matmul(
                                G,
                                normedT[:, kd, 128 * i:128 * (i + 1)],
                                normedT[:, kd, 512 * j:512 * (j + 1)],
                                start=(kd == 0), stop=(kd == KD - 1),
                            )
                        nc.scalar.activation(
                            out=relu_scr, in_=G, func=AF.Relu,
                            accum_out=cpart[:, MJ * i + j:MJ * i + j + 1])
                    nc.vector.tensor_reduce(
                        out=counter_all[:, i:i + 1],
                        in_=cpart[:, MJ * i:MJ * (i + 1)],
                        axis=mybir.AxisListType.X, op=ALU.add,
                    )
                    if i == 7:
                        csp_half(0)
                if True:
                    csp_half(1)

                # ---- merger: out = s .* (normed @ W2a) + csp @ W2b
                for i in range(NT):
                    out_t = outp.tile([128, D], F32, tag="out_t")
                    for dd in range(2):
                        A = ps_a.tile([128, 512], F32, tag="A")
                        Bp = ps_b.tile([128, 512], F32, tag="B")
                        for kd in range(KD):
                            nc.tensor.matmul(
                                A,
                                normedT[:, kd, 128 * i:128 * (i + 1)],
                                w2a[:, kd, 512 * dd:512 * (dd + 1)],
                                start=(kd == 0), stop=(kd == KD - 1),
                            )
                        for kc in range(KD):
                            nc.tensor.matmul(
                                Bp,
                                cspT[:, kc, 128 * i:128 * (i + 1)],
                                w2b[:, kc, 512 * dd:512 * (dd + 1)],
                                start=(kc == 0), stop=(kc == KD - 1),
                            )
                        t1 = t1p.tile([128, 512], F32, tag="t1")
                        nc.vector.tensor_scalar_mul(out=t1, in0=A,
                                                    scalar1=s_all[:, i:i + 1])
                        nc.vector.tensor_add(
                            out=out_t[:, 512 * dd:512 * (dd + 1)],
                            in0=t1, in1=Bp)
                    nc.sync.dma_start(out=out[128 * i:128 * (i + 1), :],
                                      in_=out_t)

            if repeat == 1:
                body(0)
            else:
                with tc.For_i(0, repeat, 1) as _:
                    body(0)

    return nc


_NC_CACHE = {}


def _get_nc(repeat: int = 1):
    key = ("nc", repeat)
    if key not in _NC_CACHE:
        _install_waitfix()
        _NC_CACHE[key] = build_kernel(repeat)
    return _NC_CACHE[key]


def kernel(data, W1, b1, W2, _trace=False, _repeat=1):
    nc = _get_nc(_repeat)
    W1 = np.ascontiguousarray(W1, dtype=np.float32).reshape(1, D)
    b1 = np.ascontiguousarray(b1, dtype=np.float32).reshape(1, D)
    W2 = np.ascontiguousarray(W2, dtype=np.float32)
    data = np.ascontiguousarray(data, dtype=np.float32)
    in_maps = [
        {"data": data[i], "W1": W1, "b1": b1, "W2": W2} for i in range(B)
    ]
    res = run_bass_kernel_spmd(nc, in_maps, core_ids=list(range(B)),
                               trace=_trace)
    outs = np.stack([res.results[i]["out"] for i in range(B)], axis=0)
    if _trace:
        return outs, res
    return outs

